# revision 7
# baseline (speedup 1.0000x reference)
"""Trainium2 Bass kernel for Conv1D(SAME) + BN + ReLU -> LocallyConnected1D + BN + ReLU.

Sharding: sequence-parallel over output positions. Core i owns output
positions [64*i, 64*i + 64) (core 7 is zero-padded past position 505).
Each core reads only its slice of local_w (the 232 MB dominant tensor),
so total HBM traffic stays at the single-read minimum. No collectives.

Host-side pre-processing folds both BatchNorms into the weights:
  y  = relu(conv(x) @ (conv_w * s1) + b1'),   s1 = g1*rsqrt(v1+eps)
  z  = relu(patches @ (local_w * s2) + b2'),  s2 = g2*rsqrt(v2+eps)
and lays x out transposed ([Cin, pos, batch]) so the conv contraction
dim is on SBUF partitions without any on-device transposes.
"""

import numpy as np

_B, _L, _CIN, _F, _K = 64, 512, 64, 128, 7
_OUT_LEN = _L - _K + 1  # 506
_NCORES = 8
_C = 64          # output positions per core (padded)
_NQ = _C + 6     # y positions needed per core: 70
_NJB = 9         # conv j-blocks of 8 -> covers j in [0, 72)
_LX = _NJB * 8 + 6  # 78: xT positions per core
_EPS = 1e-3
_WPAIR = 2       # local_w positions per DMA
_WBUFS = 8       # in-flight W tiles (2 positions each)


def _build_program(bias_en: bool):
    import concourse.mybir as mybir
    import concourse.tile as tile
    from concourse import bacc

    f32 = mybir.dt.float32
    nc = bacc.Bacc("TRN2", target_bir_lowering=False, debug=False)

    xt_d = nc.dram_tensor("xt", [_CIN, _LX * _B], f32, kind="ExternalInput")
    wc_d = nc.dram_tensor("wc", [_K, _CIN, _F], f32, kind="ExternalInput")
    b1_d = nc.dram_tensor("b1", [_F, 1], f32, kind="ExternalInput")
    wl_d = nc.dram_tensor("wl", [_C, _K, _F, _F], f32, kind="ExternalInput")
    if bias_en:
        b2_d = nc.dram_tensor("b2", [1, _C * _F], f32, kind="ExternalInput")
    z_d = nc.dram_tensor("z", [_B, _C * _F], f32, kind="ExternalOutput")

    Relu = mybir.ActivationFunctionType.Relu

    with tile.TileContext(nc) as tc:
        with (
            tc.tile_pool(name="const", bufs=1) as cpool,
            tc.tile_pool(name="xt", bufs=1) as xpool,
            tc.tile_pool(name="yt", bufs=1) as ypool,
            tc.tile_pool(name="wt", bufs=_WBUFS) as wpool,
            tc.tile_pool(name="zst", bufs=4) as zpool,
            tc.tile_pool(name="psc", bufs=2, space="PSUM") as pscpool,
            tc.tile_pool(name="psl", bufs=4, space="PSUM") as pslpool,
        ):
            # ---- constants / inputs to SBUF ----
            wc_t = cpool.tile([_CIN, _K * _F], f32)
            nc.scalar.dma_start(
                wc_t[:].rearrange("c (k f) -> c k f", k=_K),
                wc_d[:].rearrange("k c f -> c k f"),
            )
            b1_t = cpool.tile([_F, 1], f32)
            nc.scalar.dma_start(b1_t[:], b1_d[:])
            if bias_en:
                b2_t = cpool.tile([1, _C * _F], f32)
                nc.scalar.dma_start(b2_t[:], b2_d[:])
                ones_t = cpool.tile([1, _B], f32)
                nc.gpsimd.memset(ones_t[:], 1.0)

            xt_t = xpool.tile([_CIN, _LX * _B], f32)
            nxc = 4
            xch = (_LX * _B) // nxc
            for c in range(nxc):
                nc.scalar.dma_start(
                    xt_t[:, c * xch:(c + 1) * xch], xt_d[:, c * xch:(c + 1) * xch]
                )

            # ---- W stream (the big DMA): 2 positions per transfer ----
            wtiles = []
            for g in range(_C // _WPAIR):
                wt = wpool.tile([_F, _WPAIR * _K * _F], f32, tag="wt")
                nc.sync.dma_start(
                    wt[:].rearrange("f (p k n) -> f p k n", p=_WPAIR, k=_K),
                    wl_d[_WPAIR * g:_WPAIR * (g + 1)].rearrange(
                        "p k f n -> f p k n"
                    ),
                )
                wtiles.append(wt)

            # ---- conv + BN1 + ReLU -> yT [F, (j, b)] ----
            yt_t = ypool.tile([_F, _NJB * 8 * _B], f32)
            for jb in range(_NJB):
                ps = pscpool.tile([_F, 8 * _B], f32, tag="psc")
                for k in range(_K):
                    nc.tensor.matmul(
                        ps[:],
                        wc_t[:, k * _F:(k + 1) * _F],
                        xt_t[:, (8 * jb + k) * _B:(8 * jb + k + 8) * _B],
                        start=(k == 0),
                        stop=(k == _K - 1),
                    )
                nc.scalar.activation(
                    yt_t[:, jb * 8 * _B:(jb + 1) * 8 * _B], ps[:], Relu, bias=b1_t[:]
                )

            # ---- locally-connected layer ----
            # bank-blocked: positions [4t, 4t+4) share one PSUM bank and one
            # accumulation group (HW start=True zeroes the whole 2KB bank).
            # All of yT is resident, so MM order is free: iterate q and fire
            # every (p, k=q-p) pair belonging to this bank.
            for t in range(_C // 4):
                ps = pslpool.tile([_B, 4 * _F], f32, tag="psl", name=f"psl_{t}")
                mms = [
                    (q, p, q - p)
                    for q in range(4 * t, 4 * t + 10)
                    for p in range(max(4 * t, q - 6), min(4 * t + 4, q + 1))
                ]
                for i, (q, p, k) in enumerate(mms):
                    wt = wtiles[p // _WPAIR]
                    off = (p % _WPAIR) * _K * _F + k * _F
                    nc.tensor.matmul(
                        ps[:, (p - 4 * t) * _F:(p - 4 * t + 1) * _F],
                        yt_t[:, q * _B:(q + 1) * _B],
                        wt[:, off:off + _F],
                        start=(i == 0),
                        stop=(i == len(mms) - 1) and not bias_en,
                    )
                base = 4 * t
                if bias_en:
                    nc.tensor.matmul(
                        ps[:],
                        ones_t[:, :_B],
                        b2_t[:, base * _F:(base + 4) * _F],
                        start=False,
                        stop=True,
                        skip_group_check=True,
                    )
                zst = zpool.tile([_B, 4 * _F], f32, tag="zst")
                nc.scalar.activation(zst[:], ps[:], Relu)
                nc.scalar.dma_start(z_d[:, base * _F:(base + 4) * _F], zst[:])
    nc.compile()
    return nc


def _host_prepare(x, conv_w, conv_b, bn1_gamma, bn1_beta, bn1_mean, bn1_var,
                  local_w, local_b, bn2_gamma, bn2_beta, bn2_mean, bn2_var):
    f = np.float32
    x = np.asarray(x, f)
    s1 = (np.asarray(bn1_gamma, f) / np.sqrt(np.asarray(bn1_var, f) + f(_EPS))).astype(f)
    wc = (np.asarray(conv_w, f) * s1[None, None, :]).astype(f)
    b1 = (s1 * (np.asarray(conv_b, f) - np.asarray(bn1_mean, f))
          + np.asarray(bn1_beta, f)).astype(f).reshape(_F, 1)
    s2 = (np.asarray(bn2_gamma, f) / np.sqrt(np.asarray(bn2_var, f) + f(_EPS))).astype(f)
    wl = (np.asarray(local_w, f) * s2[None, None, :]).astype(f)
    b2 = (s2[None, :] * (np.asarray(local_b, f) - np.asarray(bn2_mean, f)[None, :])
          + np.asarray(bn2_beta, f)[None, :]).astype(f)

    bias_en = bool(np.any(b2))

    # zero-pad so every core sees the same shapes
    npad = _NCORES * _C  # 512
    wl_pad = np.zeros((npad, _K * _F, _F), f)
    wl_pad[:_OUT_LEN] = wl
    b2_pad = np.zeros((npad, _F), f)
    b2_pad[:_OUT_LEN] = b2
    # x padded for SAME conv + per-core halo: xpad[j] = x[j-3], j in [0, 512+3+16)
    xpad = np.zeros((_B, _L + 3 + 16, _CIN), f)
    xpad[:, 3:3 + _L] = x

    in_maps = []
    for i in range(_NCORES):
        p0 = _C * i
        xs = xpad[:, p0:p0 + _LX, :]                      # [B, LX, CIN]
        xt = np.ascontiguousarray(xs.transpose(2, 1, 0)).reshape(_CIN, _LX * _B)
        wli = np.ascontiguousarray(
            wl_pad[p0:p0 + _C].reshape(_C, _K, _F, _F))
        m = {"xt": xt, "wc": wc, "b1": b1, "wl": wli}
        if bias_en:
            m["b2"] = np.ascontiguousarray(b2_pad[p0:p0 + _C].reshape(1, _C * _F))
        in_maps.append(m)
    return in_maps, bias_en


def _assemble(results):
    f = np.float32
    z = np.empty((_B, _OUT_LEN, _F), f)
    for i in range(_NCORES):
        p0 = _C * i
        zi = np.asarray(results[i]["z"], f).reshape(_B, _C, _F)
        n = min(_C, _OUT_LEN - p0)
        z[:, p0:p0 + n] = zi[:, :n]
    return z


def kernel(**inputs) -> np.ndarray:
    from concourse.bass_utils import run_bass_kernel_spmd

    in_maps, bias_en = _host_prepare(**inputs)
    nc = _build_program(bias_en)
    res = run_bass_kernel_spmd(nc, in_maps, list(range(_NCORES)))
    return _assemble(res.results)


# revision 12
# speedup vs baseline: 1.0994x; 1.0994x over previous
"""Trainium2 Bass kernel for Conv1D(SAME) + BN + ReLU -> LocallyConnected1D + BN + ReLU.

Sharding: sequence-parallel over output positions. Core i owns output
positions [64*i, 64*i + 64) (core 7 is zero-padded past position 505).
Each core reads only its slice of local_w (the 232 MB dominant tensor),
so total HBM traffic stays at the single-read minimum. No collectives.

Host-side pre-processing folds both BatchNorms into the weights:
  y  = relu(conv(x) @ (conv_w * s1) + b1'),   s1 = g1*rsqrt(v1+eps)
  z  = relu(patches @ (local_w * s2) + b2'),  s2 = g2*rsqrt(v2+eps)
and lays x out transposed ([Cin, pos, batch]) so the conv contraction
dim is on SBUF partitions without any on-device transposes.

local_w is pre-interleaved per position-pair so that the two chunks
needed at a given y-position q are adjacent in SBUF, giving N=256
matmuls (required for full-rate float32r streaming on the PE).
PSUM sub-slots are pair-swapped ([p1, p0, p3, p2]); the host unpermutes.
"""

import numpy as np

_B, _L, _CIN, _F, _K = 64, 512, 64, 128, 7
_OUT_LEN = _L - _K + 1  # 506
_NCORES = 8
_C = 64              # output positions per core (padded)
_NPAIR = _C // 2     # 32 position pairs
_NJB = 9             # conv j-blocks of 8 -> covers y positions [0, 72)
_LX = _NJB * 8 + 6   # 78 x positions per core (with halo + SAME pad)
_EPS = 1e-3
_WBUFS = 8           # in-flight local_w pair tiles
_MODE = "f32r"       # "f32" | "f32r" | "bf16"


def _np_dt(mode):
    if mode == "bf16":
        import ml_dtypes
        return ml_dtypes.bfloat16
    return np.float32


def _build_program(bias_en: bool, mode: str | None = None):
    mode = mode or _MODE
    import concourse.mybir as mybir
    import concourse.tile as tile
    from concourse import bacc

    f32 = mybir.dt.float32
    # storage dtype for matmul operands: walrus requires FP32r consumers to
    # read locations *written* as FP32r, so declare end-to-end, no bitcast.
    dt_st = {"bf16": mybir.dt.bfloat16, "f32r": mybir.dt.float32r}.get(mode, f32)
    cast = lambda ap: ap

    nc = bacc.Bacc("TRN2", target_bir_lowering=False, debug=False)

    xt_d = nc.dram_tensor("xt", [_CIN, _LX * _B], dt_st, kind="ExternalInput")
    wc_d = nc.dram_tensor("wc", [_K, _CIN, _F], dt_st, kind="ExternalInput")
    b1_d = nc.dram_tensor("b1", [_F, 1], f32, kind="ExternalInput")
    wl_d = nc.dram_tensor("wl", [_NPAIR, 2 * _K, _F, _F], dt_st, kind="ExternalInput")
    if bias_en:
        b2_d = nc.dram_tensor("b2", [1, _C * _F], f32, kind="ExternalInput")
    z_d = nc.dram_tensor("z", [_B, _C * _F], f32, kind="ExternalOutput")

    Relu = mybir.ActivationFunctionType.Relu

    with tile.TileContext(nc) as tc:
        with (
            tc.tile_pool(name="const", bufs=1) as cpool,
            tc.tile_pool(name="xt", bufs=1) as xpool,
            tc.tile_pool(name="yt", bufs=1) as ypool,
            tc.tile_pool(name="wt", bufs=_WBUFS) as wpool,
            tc.tile_pool(name="zst", bufs=4) as zpool,
            tc.tile_pool(name="psc", bufs=2, space="PSUM") as pscpool,
            tc.tile_pool(name="psl", bufs=4, space="PSUM") as pslpool,
        ):
            # ---- constants / inputs to SBUF ----
            wc_t = cpool.tile([_CIN, _K * _F], dt_st)
            nc.scalar.dma_start(
                wc_t[:].rearrange("c (k f) -> c k f", k=_K),
                wc_d[:].rearrange("k c f -> c k f"),
            )
            b1_t = cpool.tile([_F, 1], f32)
            nc.scalar.dma_start(b1_t[:], b1_d[:])
            if bias_en:
                b2_t = cpool.tile([1, _C * _F], f32)
                nc.scalar.dma_start(b2_t[:], b2_d[:])
                ones_t = cpool.tile([1, _B], f32)
                nc.gpsimd.memset(ones_t[:], 1.0)

            xt_t = xpool.tile([_CIN, _LX * _B], dt_st)
            nxc = 4
            xch = (_LX * _B) // nxc
            for c in range(nxc):
                nc.scalar.dma_start(
                    xt_t[:, c * xch:(c + 1) * xch], xt_d[:, c * xch:(c + 1) * xch]
                )

            # ---- W stream (the big DMA): one position-pair per transfer ----
            wtiles = []
            for g in range(_NPAIR):
                wt = wpool.tile([_F, 2 * _K * _F], dt_st, tag="wt", name=f"wt{g}")
                nc.sync.dma_start(
                    wt[:].rearrange("f (c n) -> f c n", c=2 * _K),
                    wl_d[g].rearrange("c f n -> f c n"),
                )
                wtiles.append(wt)

            # ---- conv + BN1 + ReLU -> yT [F, (j, b)] ----
            yt_t = ypool.tile([_F, _NJB * 8 * _B], dt_st)
            for jb in range(_NJB):
                ps = pscpool.tile([_F, 8 * _B], f32, tag="psc", name=f"psc{jb}")
                for k in range(_K):
                    nc.tensor.matmul(
                        ps[:],
                        cast(wc_t[:, k * _F:(k + 1) * _F]),
                        cast(xt_t[:, (8 * jb + k) * _B:(8 * jb + k + 8) * _B]),
                        start=(k == 0),
                        stop=(k == _K - 1),
                    )
                nc.scalar.activation(
                    yt_t[:, jb * 8 * _B:(jb + 1) * 8 * _B], ps[:], Relu, bias=b1_t[:]
                )

            # ---- locally-connected layer ----
            # bank-blocked: positions [4t, 4t+4) share one PSUM bank and one
            # accumulation group (HW start=True zeroes the whole 2KB bank).
            # wl cols: c = 2k + (p%2); at stationary q the active chunks of a
            # pair are adjacent -> one N=256 matmul. PSUM sub-slot of local
            # position j is j^1 (pair-swapped); host unpermutes.
            for t in range(_C // 4):
                ps = pslpool.tile([_B, 4 * _F], f32, tag="psl", name=f"psl{t}")
                # singles first: the start=True MM marks the whole 2KB bank
                # pending; the other three singles land in fully-pending
                # slots; every later paired MM then touches only
                # already-written bytes (uniform accumulate).
                mms = [  # (q, g, col_lo, ncols, out_lo)
                    (4 * t, 2 * t, 0, 1, 1),
                    (4 * t + _K, 2 * t, 2 * _K - 1, 1, 0),
                    (4 * t + 2, 2 * t + 1, 0, 1, 3),
                    (4 * t + 2 + _K, 2 * t + 1, 2 * _K - 1, 1, 2),
                ]
                for q in range(4 * t, 4 * t + 10):
                    for g in (2 * t, 2 * t + 1):
                        ke, ko = q - 2 * g, q - 2 * g - 1
                        if 0 <= ko and ke < _K:          # both chunks active
                            mms.append((q, g, 2 * ke - 1, 2, 2 * g - 4 * t))
                for i, (q, g, c0, ncol, u0) in enumerate(mms):
                    nc.tensor.matmul(
                        ps[:, u0 * _F:(u0 + ncol) * _F],
                        cast(yt_t[:, q * _B:(q + 1) * _B]),
                        cast(wtiles[g][:, c0 * _F:(c0 + ncol) * _F]),
                        start=(i == 0),
                        stop=(i == len(mms) - 1) and not bias_en,
                    )
                base = 4 * t
                if bias_en:
                    nc.tensor.matmul(
                        ps[:],
                        cast(ones_t[:, :_B]),
                        cast(b2_t[:, base * _F:(base + 4) * _F]),
                        start=False,
                        stop=True,
                        skip_group_check=True,
                    )
                zst = zpool.tile([_B, 4 * _F], f32, tag="zst", name=f"zst{t}")
                nc.scalar.activation(zst[:], ps[:], Relu)
                nc.scalar.dma_start(z_d[:, base * _F:(base + 4) * _F], zst[:])
    nc.compile()
    return nc


def _host_prepare(x, conv_w, conv_b, bn1_gamma, bn1_beta, bn1_mean, bn1_var,
                  local_w, local_b, bn2_gamma, bn2_beta, bn2_mean, bn2_var,
                  mode: str | None = None):
    mode = mode or _MODE
    f = np.float32
    dt = _np_dt(mode)
    x = np.asarray(x, f)
    s1 = (np.asarray(bn1_gamma, f) / np.sqrt(np.asarray(bn1_var, f) + f(_EPS))).astype(f)
    wc = (np.asarray(conv_w, f) * s1[None, None, :]).astype(dt)
    b1 = (s1 * (np.asarray(conv_b, f) - np.asarray(bn1_mean, f))
          + np.asarray(bn1_beta, f)).astype(f).reshape(_F, 1)
    s2 = (np.asarray(bn2_gamma, f) / np.sqrt(np.asarray(bn2_var, f) + f(_EPS))).astype(f)
    wl = (np.asarray(local_w, f) * s2[None, None, :]).astype(f)
    b2 = (s2[None, :] * (np.asarray(local_b, f) - np.asarray(bn2_mean, f)[None, :])
          + np.asarray(bn2_beta, f)[None, :]).astype(f)

    bias_en = bool(np.any(b2))

    npad = _NCORES * _C  # 512
    # pair-interleaved local_w: [pair, c=2k+(p%2), f, n]
    wl_pad = np.zeros((npad, _K, _F, _F), f)
    wl_pad[:_OUT_LEN] = wl.reshape(_OUT_LEN, _K, _F, _F)
    wl_pi = np.ascontiguousarray(
        wl_pad.reshape(npad // 2, 2, _K, _F, _F).transpose(0, 2, 1, 3, 4)
    ).reshape(npad // 2, 2 * _K, _F, _F).astype(dt)

    perm = np.arange(_C) ^ 1  # pair-swap (self-inverse)
    b2_pad = np.zeros((npad, _F), f)
    b2_pad[:_OUT_LEN] = b2

    # x padded for SAME conv + per-core halo: xpad[:, j] = x[:, j-3]
    xpad = np.zeros((_B, _L + 3 + 16, _CIN), f)
    xpad[:, 3:3 + _L] = x
    xpad = xpad.astype(dt)

    in_maps = []
    for i in range(_NCORES):
        p0 = _C * i
        xs = xpad[:, p0:p0 + _LX, :]                      # [B, LX, CIN]
        xt = np.ascontiguousarray(xs.transpose(2, 1, 0)).reshape(_CIN, _LX * _B)
        wli = np.ascontiguousarray(wl_pi[p0 // 2:p0 // 2 + _NPAIR])
        m = {"xt": xt, "wc": wc, "b1": b1, "wl": wli}
        if bias_en:
            m["b2"] = np.ascontiguousarray(
                b2_pad[p0:p0 + _C][perm].reshape(1, _C * _F))
        in_maps.append(m)
    return in_maps, bias_en


def _assemble(results):
    f = np.float32
    perm = np.arange(_C) ^ 1
    z = np.empty((_B, _OUT_LEN, _F), f)
    for i in range(_NCORES):
        p0 = _C * i
        zi = np.asarray(results[i]["z"], f).reshape(_B, _C, _F)[:, perm]
        n = min(_C, _OUT_LEN - p0)
        z[:, p0:p0 + n] = zi[:, :n]
    return z


def kernel(**inputs) -> np.ndarray:
    from concourse.bass_utils import run_bass_kernel_spmd

    in_maps, bias_en = _host_prepare(**inputs)
    nc = _build_program(bias_en)
    res = run_bass_kernel_spmd(nc, in_maps, list(range(_NCORES)))
    return _assemble(res.results)
